# revision 1
# baseline (speedup 1.0000x reference)
"""Causal RBF (non-softmax) attention on 8 Trainium2 NeuronCores.

Problem: q,k,v [B=2, H=16, N=2048, D=128] f32.
  logits = 2s*q@k^T - s*||q||^2 - s*||k||^2   (s = 1/sqrt(D))
  p = exp(logits) with causal mask; out = p @ v      (no softmax normalization)

Sharding: B*H = 32 heads -> 4 heads per core, fully independent.
Host folds 2s into qT, ek into v, eq applied to output rows on host, so the
device computes only:  ST = kt_blk.T @ qt -> Exp -> tri-mask -> @ V'.

Design (measured ~89.5us NEFF exec vs 97.4us for the f32r single-stream
version; rel err 4.9e-3 vs the 2e-2 gate):
  - bf16 operands everywhere: halves DMA bytes and SBUF, enables FWL fast
    weight loads on the PE (LDWEIGHTS 93ns vs 172ns), same 1 col/cycle
    matmul rate as f32r.
  - Two interleaved streams (heads 0-1 vs heads 2-3), each with a
    single-buffered 3-bank PSUM ST tile + 1-bank OT accumulator (8 banks
    exactly).  While stream A's EXP runs on ACT, stream B's matmuls keep
    the PE busy and vice versa: ACT (the roofline engine: 1.2GHz, 1
    elem/lane/cycle, ~180cyc/instr overhead -> 66us of EXP per core) runs
    with <2us of gaps outside the DMA ramp.
  - Exact 1280-col diagonal layout (b0:512 | b1:384 | b3:128 | b2:256),
    flat one-AP EXPs, 3 tri-mask DVE multiplies (128/128/256, one [tri|tri]
    constant) instead of 4.
  - Diag-first item order per supertile so the final item is a plain full
    chunk (short exp->pv->cast->dma tail); per-supertile OT closes with a
    DVE cast to bf16 and a 1-bank-free handoff.
  - DMA discipline: each dma_start costs ~650ns of issuing-queue time
    regardless of size, and a consumer of transfer #k waits for #k+3
    (completion sem shared across HW-DGE queues).  First head per stream:
    8 chunked transfers ordered qk0,qk1,v0,v1,qk2,v2,qk3,v3 on its own
    queue (sync for A, gpsimd for B, cmask first on gpsimd) so the skew is
    absorbed by transfers needed later; subsequent heads prefetch a full
    head ahead in 2 whole-head transfers; output DMAs go to the opposite
    stream's queue.

Device layouts (per head):
  qk [2, 128(d), 2048] bf16: plane 0 = qT scaled by 2s, plane 1 = kT
  v  [2048(n), 128(d)] bf16 (ek-scaled, natural)
Output written transposed, OT [128(d), 2048(m)] bf16; host transposes back
and applies eq.
"""

import math
import sys
import time

import numpy as np

sys.path.insert(0, "/opt/trn_rl_repo")

import ml_dtypes

import concourse.mybir as mybir
import concourse.tile as tile
from concourse import bacc, bass_utils

F32 = mybir.dt.float32
BF16 = mybir.dt.bfloat16
EXP = mybir.ActivationFunctionType.Exp
BFNP = ml_dtypes.bfloat16

B, H, N, D = 2, 16, 2048, 128
SM = 1.0 / math.sqrt(D)
P = 128
NCORES = 8
HPC = (B * H) // NCORES  # heads per core
MW = 512                 # m (query) super-tile width
MI = N // MW             # super tiles per head

# diag flat layout within the 3-bank (1536-col) ST tile:
#   [0:512)=b0  [512:896)=b1  [896:1024)=b3  [1024:1280)=b2
DIAG = [  # (b, flat_lo, flat_hi, m_lo) ; m range is [m_lo : 512) of the supertile
    (0, 0, 512, 0),
    (1, 512, 896, 128),
    (3, 896, 1024, 384),
    (2, 1024, 1280, 256),
]


def _emit_body(tc, qk, v, cmask, out, hpc, n):
    nc = tc.nc
    from contextlib import ExitStack

    with ExitStack() as ctx:
        const = ctx.enter_context(tc.tile_pool(name="const", bufs=1))
        io_pool = ctx.enter_context(tc.tile_pool(name="io", bufs=1))
        st_pool = ctx.enter_context(tc.tile_pool(name="st", bufs=1, space="PSUM"))
        ot_pool = ctx.enter_context(tc.tile_pool(name="ot", bufs=1, space="PSUM"))
        pt_pool = ctx.enter_context(tc.tile_pool(name="pt", bufs=3))
        osb_pool = ctx.enter_context(tc.tile_pool(name="osb", bufs=2))

        # streams: A = heads [0, 1] on sync DMA queue, B = heads [2, 3] on
        # gpsimd queue.
        streams = [
            {"name": "A", "heads": [0, 1], "dma": nc.sync, "odma": nc.gpsimd},
            {"name": "B", "heads": [2, 3], "dma": nc.gpsimd, "odma": nc.sync},
        ]

        head_tiles = {}
        masks = {}

        def emit_loads(s, h, first=False):
            # Each DMA has a ~650ns fixed cost on its queue, and a consumer
            # of transfer #k conservatively waits for #k+3 (completion sem
            # shared across the HW queues the engine fans out to).  So:
            # prefetched heads load in 2 whole-head transfers; the first
            # head of each stream uses 8 chunked transfers ordered so the
            # +3 skew is absorbed by transfers needed later anyway.
            eng = s["dma"]
            qkc = io_pool.tile([P, 2, n], BF16, tag=f"qk{h}")
            vc = io_pool.tile([P, n // P, P], BF16, tag=f"v{h}")
            head_tiles[h] = (qkc, vc)
            if not first:
                eng.dma_start(qkc[:], qk[h].rearrange("t d m -> d t m"))
                eng.dma_start(
                    vc[:], v[h].rearrange("(nb p) d -> p nb d", p=P)
                )
                return

            def load_qk(c):
                eng.dma_start(
                    qkc[:, :, c * MW : (c + 1) * MW],
                    qk[h, :, :, c * MW : (c + 1) * MW].rearrange(
                        "t d m -> d t m"
                    ),
                )

            def load_v(c):
                eng.dma_start(
                    vc[:, c * 4 : (c + 1) * 4, :],
                    v[h, c * MW : (c + 1) * MW].rearrange(
                        "(nb p) d -> p nb d", p=P
                    ),
                )

            load_qk(0)
            load_qk(1)
            load_v(0)
            load_v(1)
            load_qk(2)
            load_v(2)
            load_qk(3)
            load_v(3)

        def kt_blk(h, j):
            return head_tiles[h][0][:, 1, j * P : (j + 1) * P]

        def v_blk(h, j):
            return head_tiles[h][1][:, j, :]

        # per-stream work list: ("full", h, i, [j...]) | ("diag", h, i)
        # diag first within each supertile: the stream's final item is then a
        # plain full chunk, shortening the end-of-kernel exp->mask->pv tail.
        def build_work(heads):
            items = []  # (kind, h, i, js_or_None, is_last_of_supertile)
            for h in heads:
                for i in range(MI):
                    fullb = list(range(4 * i))
                    items.append(("diag", h, i, None, not fullb))
                    for c0 in range(0, len(fullb), 3):
                        items.append(
                            ("full", h, i, fullb[c0 : c0 + 3],
                             c0 + 3 >= len(fullb))
                        )
            return items

        work_per_stream = [build_work(s["heads"]) for s in streams]
        assert len(work_per_stream[0]) == len(work_per_stream[1])
        # interleave A/B
        work = []
        for wa, wb in zip(*work_per_stream):
            work.append((0, wa))
            work.append((1, wb))

        ustate = {}  # (snum, i-key) -> dict(ot=..., first=...)
        pend = {}    # k -> pt tile

        def st_exp(k):
            snum, item = work[k]
            s = streams[snum]
            kind, h, i, js, last = item
            if (
                kind == "full"
                and i == 1
                and js[:1] == [0]
                and h + 1 in s["heads"]
            ):
                emit_loads(s, h + 1)  # prefetch next head, a full head ahead
            qs = head_tiles[h][0][:, 0, i * MW : (i + 1) * MW]
            st = st_pool.tile([P, 3 * MW], F32, tag=f"st{snum}")
            pt = pt_pool.tile([P, 3 * MW], BF16, tag=f"pt{snum}")
            if kind == "full":
                for idx, j in enumerate(js):
                    nc.tensor.matmul(
                        st[:, idx * MW : (idx + 1) * MW],
                        lhsT=kt_blk(h, j), rhs=qs[:],
                        start=True, stop=True,
                    )
                nc.scalar.activation(
                    pt[:, 0 : len(js) * MW], st[:, 0 : len(js) * MW], EXP
                )
            else:
                jb = 4 * i
                for b, lo, hi, mlo in DIAG:
                    nc.tensor.matmul(
                        st[:, lo:hi], lhsT=kt_blk(h, jb + b),
                        rhs=qs[:, mlo:MW], start=True, stop=True,
                    )
                nc.scalar.activation(pt[:, 0:1280], st[:, 0:1280], EXP)
            pend[k] = pt

        def finish(k):
            snum, item = work[k]
            kind, h, i, js, last = item
            pt = pend.pop(k)
            u = ustate.get((snum, h, i))
            if u is None:
                ot_tile = ot_pool.tile([P, MW], F32, tag=f"ot{snum}")
                u = ustate[(snum, h, i)] = {"ot": ot_tile, "first": True}
            ot = u["ot"]

            def pv(j, rhs, osl, stop=False):
                nc.tensor.matmul(osl, lhsT=v_blk(h, j), rhs=rhs,
                                 start=u["first"], stop=stop)
                u["first"] = False

            if kind == "full":
                for idx, j in enumerate(js):
                    pv(j, pt[:, idx * MW : (idx + 1) * MW], ot[:, :],
                       stop=(last and idx == len(js) - 1))
            else:
                jb = 4 * i
                # causal tri-masks: b0 head, b1 head, b3|b2 heads (contiguous)
                nc.vector.tensor_mul(pt[:, 0:P], pt[:, 0:P], masks["tri"])
                nc.vector.tensor_mul(
                    pt[:, MW : MW + P], pt[:, MW : MW + P], masks["tri"]
                )
                nc.vector.tensor_mul(
                    pt[:, 896 : 896 + 2 * P], pt[:, 896 : 896 + 2 * P],
                    masks["tritri"],
                )
                for b, lo, hi, mlo in DIAG:
                    pv(jb + b, pt[:, lo:hi], ot[:, mlo:MW],
                       stop=(last and b == 2))
            if last:
                # close out the supertile
                out_sb = osb_pool.tile([P, MW], BF16, tag=f"osb{snum}")
                nc.vector.tensor_copy(out_sb[:], ot[:])
                streams[snum]["odma"].dma_start(
                    out[h, :, i * MW : (i + 1) * MW], out_sb[:]
                )

        # cmask [P, 256] = [tri | tri]; tri[p, c] = 1 where c >= p.
        # First transfer on the gpsimd queue: ready (with the +3-transfer
        # wait skew) well before the first diag masks on either stream.
        cm_sb = const.tile([P, 2 * P], BF16)
        nc.gpsimd.dma_start(cm_sb[:], cmask[:])
        tri = cm_sb[:, 0:P]
        tritri = cm_sb[:, 0 : 2 * P]
        masks["tri"] = tri
        masks["tritri"] = tritri
        emit_loads(streams[0], streams[0]["heads"][0], first=True)
        emit_loads(streams[1], streams[1]["heads"][0], first=True)
        st_exp(0)
        st_exp(1)
        for k in range(len(work)):
            if k + 2 < len(work):
                st_exp(k + 2)
            finish(k)


def _build(hpc=HPC, n=N):
    nc = bacc.Bacc(
        "TRN2", target_bir_lowering=False, debug=False, num_devices=NCORES
    )
    qk = nc.dram_tensor("qk", [hpc, 2, P, n], BF16, kind="ExternalInput").ap()
    v = nc.dram_tensor("v", [hpc, n, P], BF16, kind="ExternalInput").ap()
    cmask = nc.dram_tensor("cmask", [P, 2 * P], BF16, kind="ExternalInput").ap()
    out = nc.dram_tensor("out", [hpc, P, n], BF16, kind="ExternalOutput").ap()
    with tile.TileContext(nc) as tc:
        _emit_body(tc, qk, v, cmask, out, hpc, n)
    nc.compile()
    return nc


_NC_CACHE = {}


def _get_nc():
    if "nc" not in _NC_CACHE:
        _NC_CACHE["nc"] = _build()
    return _NC_CACHE["nc"]


def _make_mask():
    # cmask [P, 256] = [tri | tri], tri[p, c] = 1 where c >= p
    c = np.arange(P)[None, :]
    p = np.arange(P)[:, None]
    tri = (c >= p).astype(np.float32)
    return np.concatenate([tri, tri], axis=1).astype(BFNP)


def _prep(q, k, v):
    """Host-side reshaping/folding. Returns per-core in_maps and eq for post."""
    q = np.asarray(q, dtype=np.float32).reshape(B * H, N, D)
    k = np.asarray(k, dtype=np.float32).reshape(B * H, N, D)
    v = np.asarray(v, dtype=np.float32).reshape(B * H, N, D)

    qT = (np.ascontiguousarray(q.transpose(0, 2, 1)) * np.float32(2.0 * SM)).astype(BFNP)
    kT = np.ascontiguousarray(k.transpose(0, 2, 1)).astype(BFNP)
    ek = np.exp(np.float32(-SM) * np.einsum("hnd,hnd->hn", k, k)).astype(np.float32)
    eq = np.exp(np.float32(-SM) * np.einsum("hnd,hnd->hn", q, q)).astype(np.float32)
    vs = (v * ek[:, :, None]).astype(BFNP)

    mask = _make_mask()
    qkT = np.ascontiguousarray(np.stack([qT, kT], axis=1))  # [BH, 2, D, N]
    in_maps = []
    for c in range(NCORES):
        s = slice(c * HPC, (c + 1) * HPC)
        in_maps.append(
            {
                "qk": np.ascontiguousarray(qkT[s]),
                "v": np.ascontiguousarray(vs[s]),
                "cmask": mask,
            }
        )
    return in_maps, eq


def _run(in_maps, trace=False):
    nc = _get_nc()
    res = bass_utils.run_bass_kernel_spmd(
        nc, in_maps, core_ids=list(range(NCORES)), trace=trace
    )
    return res


def _post(res_list, eq):
    # res_list: per-core dicts with "out" [HPC, 128(d), N(m)] bf16
    ot = np.concatenate(
        [r["out"].astype(np.float32) for r in res_list], axis=0
    )  # [B*H, D, N]
    o = ot.transpose(0, 2, 1) * eq[:, :, None]  # [B*H, N, D]
    return np.ascontiguousarray(o.reshape(B, H, N, D).astype(np.float32))


def kernel(q, k, v):
    in_maps, eq = _prep(q, k, v)
    last_err = None
    for attempt in range(3):
        try:
            res = _run(in_maps, trace=False)
            return _post(res.results, eq)
        except Exception as e:  # axon/NRT first-run flakiness: retry
            last_err = e
            time.sleep(2.0)
    raise last_err

